# revision 50
# baseline (speedup 1.0000x reference)
"""CantorAttention TRN2 kernel: 8-core SPMD Bass/Tile, sequence-sharded.

Math (reference): qkv = x @ W_qkv + b; per-head sparse attention over the
128 nearest neighbours in 1-D cantor space; out = attn_out @ W_out + b_out.

Key structural facts exploited:
  * top_k(-|p_i - p_j|) sets are contiguous windows in sorted-position order,
    so after permuting tokens by sorted cantor position the sparse attention
    becomes BANDED attention: each query block only sees a narrow aligned
    band of keys, with a per-(query,key) 0/1 mask reproducing the exact
    reference top-k set (host-computed from cantor_positions only).
  * exp() needs no running-max: |score*scale| < ~3 for this distribution,
    so softmax = exp(s)*mask with a ones-column fused into V producing the
    denominators inside the AV matmul.

Sharding (8 cores): FULL sequence sharding, zero collectives. Core c owns
sorted-token rows [256c, 256c+256) (query blocks 2c, 2c+1). It computes
  * Q for its own 256 tokens, all 16 heads        (x_own  @ W_q)
  * K,V for its NBC*128-token key band, all heads (x_band @ W_{k,v})
  * banded masked attention for its 2 query blocks x 16 heads
  * the full out-projection for its 256 tokens    (o @ W_out + b_out)
and writes a [256, 1024] f32 slice; the host concatenates and un-sorts.
The K/V band work is ~2x redundant vs head-sharding, but it removes the
AllToAll entirely (the cost model charges a fixed ~15us per collective,
serialized on an exclusive device -- two of them dominated the baseline).

The program is identical on all cores (SPMD): per-core geometry (band
start, neighbour sets) lives entirely in the per-core INPUT data (x slices
and the 0/1 mask); the compiled program depends only on NBC (band chunks).

Scheduling notes (cost-model driven):
  * The PE p-state ramp makes idle gaps expensive (post-gap matmuls run
    2-3.7x slow until 3us of continuous busy), so every phase is ordered so
    the PE never waits: weights stream per-cc ahead of consumption, and the
    attention epilogue is restructured to be latency-tolerant.
  * AV is computed TRANSPOSED per head: out[65, 256] = (V+ones)^T @ ptm for
    both query blocks at once; row 64 is the softmax denominator. Its
    reciprocal row is broadcast to rows 64:128 of the same PSUM slot by a
    tiny K=1 matmul with a ones-row, and one DVE multiply writes the
    normalized o^T straight into the out-projection's lhsT layout -- no PE
    transposes, no per-item DVE round-trip stalls.
  * Small loads go through the Pool/SWDGE queue or late in the sync queue
    so they never steal HWDGE/DMA slots from the weight stream.

All data-dependent indexing (sort permutation, band offsets, masks) is
resolved on the host; the device program is a fixed dense pipeline.
"""

import numpy as np
import ml_dtypes

import concourse.bass as bass
from concourse import bacc
import concourse.mybir as mybir
import concourse.tile as tile
from concourse.bass import ts
from concourse.bass_utils import run_bass_kernel_spmd

BF16 = ml_dtypes.bfloat16

# Problem constants (hardcoded per contract).
N = 2048          # sequence length
D = 1024          # model dim
H = 16            # heads
HD = 64           # head dim
K_NEIGH = 128     # neighbours per query
SCALE = 1.0 / np.sqrt(HD)
NCORES = 8
TPC = N // NCORES            # tokens per core = 256 (2 query blocks)
NBLK_PC = TPC // 128         # query blocks per core = 2
CC = D // 128                # 128-channel chunks per projection = 8
KT = D // 128                # contraction tiles = 8
MAX_NBC = 6                  # hard cap on 128-wide band chunks per core

# Results of the most recent run (exec_time_ns etc.) for the test harness.
LAST_RESULT = None


def _build_program(NBT):
    """Build the SPMD Bass program (band width NBT tokens per core, 64-mult)."""
    f32 = mybir.dt.float32
    bf16 = mybir.dt.bfloat16
    NBC = (NBT + 127) // 128     # key chunks (last may be half)
    VREM = NBT - (NBT // 128) * 128  # tokens in the partial last chunk

    nc = bacc.Bacc(None, target_bir_lowering=False, num_devices=NCORES)
    xb_d = nc.declare_dram_parameter("xb", [D, NBT], bf16, isOutput=False)
    # wq/wk host layout: [cc, 128, KT*128] so each cc slice is one big-elem DMA
    wq_d = nc.declare_dram_parameter("wq", [CC, 128, KT * 128], bf16, isOutput=False)
    wk_d = nc.declare_dram_parameter("wk", [CC, 128, KT * 128], bf16, isOutput=False)
    wv_d = nc.declare_dram_parameter("wv", [D, D], bf16, isOutput=False)
    wout_d = nc.declare_dram_parameter("wout", [D, D], bf16, isOutput=False)
    mask_d = nc.declare_dram_parameter("mask", [128, NBC, TPC], bf16, isOutput=False)
    bq_d = nc.declare_dram_parameter("bq", [128, CC], f32, isOutput=False)
    bk_d = nc.declare_dram_parameter("bk", [128, CC], f32, isOutput=False)
    bv_d = nc.declare_dram_parameter("bv", [D], f32, isOutput=False)
    boutr_d = nc.declare_dram_parameter("boutr", [1, D], bf16, isOutput=False)
    out_d = nc.declare_dram_parameter("out", [TPC, D], bf16, isOutput=True)

    Exp = mybir.ActivationFunctionType.Exp
    Ident = mybir.ActivationFunctionType.Identity

    with tile.TileContext(nc) as tc:
        with (
            tc.tile_pool(name="const", bufs=1) as const,
            tc.tile_pool(name="pt", bufs=3) as ptp,
            tc.tile_pool(name="ptm", bufs=H) as ptmp,
            tc.tile_pool(name="small", bufs=8) as smallp,
            tc.tile_pool(name="psum_big", bufs=3, space="PSUM") as ps_bigp,
            tc.tile_pool(name="psum_s", bufs=2, space="PSUM") as ps_sp,
            tc.tile_pool(name="psum_av", bufs=3, space="PSUM") as ps_avp,
        ):
            # ---- input streams --------------------------------------------------
            # Main stream on the sync DGE queue: this order IS the arrival
            # order (FIFO queue, exclusive DMA-engines device). mask/bvb/boutb
            # ride late in the same stream so they never delay weights; their
            # consumers run much later anyway.
            # The band is host-rotated so the core's own 256 tokens occupy
            # columns 0:TPC -- the Q projection reads them straight out of
            # xb, so there is no separate xq stream.
            # K runs first: only xb (1MB) + wk must land before the PE's
            # first real phase, so compute starts ~4.4us in instead of ~10.
            xb_sb = const.tile([128, KT, NBT], bf16)
            xb_r = xb_d[:].rearrange("(k p) t -> p k t", p=128)
            wk_sb = const.tile([128, CC, KT * 128], bf16)
            nc.sync.dma_start(xb_sb[:, 0:4, :], xb_r[:, 0:4, :])
            nc.sync.dma_start(wk_sb[:, 0, 0:512], wk_d[0][:, 0:512])
            nc.sync.dma_start(xb_sb[:, 4:8, :], xb_r[:, 4:8, :])
            nc.sync.dma_start(wk_sb[:, 0, 512:], wk_d[0][:, 512:])
            for cc in range(1, CC):
                nc.sync.dma_start(wk_sb[:, cc, :], wk_d[cc])
            mask_sb = const.tile([128, NBC, TPC], bf16)
            nc.sync.dma_start(mask_sb, mask_d[:])
            wq_sb = const.tile([128, CC, KT * 128], bf16)
            for cc in range(CC):
                nc.sync.dma_start(wq_sb[:, cc, :], wq_d[cc])
            wv_sb = const.tile([128, KT, D], bf16)
            for kt in range(KT):
                nc.sync.dma_start(wv_sb[:, kt, :], wv_d[ts(kt, 128), :])
            bvb_sb = const.tile([128, D], f32)
            nc.sync.dma_start(
                bvb_sb, bv_d[:].rearrange("(a c) -> a c", a=1).to_broadcast([128, D])
            )
            wout_sb = const.tile([128, KT, D], bf16)
            for kt in range(KT):
                nc.sync.dma_start(wout_sb[:, kt, :], wout_d[ts(kt, 128), :])
            boutr_sb = const.tile([1, D], bf16)
            nc.sync.dma_start(boutr_sb, boutr_d[:])

            # Tiny early loads via Pool/SWDGE: no HWDGE contention, ~60ns of
            # DMA-engines time each.
            bq_sb = const.tile([128, CC], f32)
            nc.gpsimd.dma_start(bq_sb, bq_d[:])
            bk_sb = const.tile([128, CC], f32)
            nc.gpsimd.dma_start(bk_sb, bk_d[:])

            # ---- working tiles --------------------------------------------------
            q_sb = const.tile([128, CC, TPC], bf16)       # [chan%128, cc, tok]
            k_sb = const.tile([128, CC, NBC * 128], bf16)  # [chan%128, cc, band tok]
            v_sb = const.tile([128, NBC, H, HD + 1], bf16)  # [tok%128, tc, h, hd+1]
            oT_sb = const.tile([128, CC, TPC], bf16)      # [chan%128, cc, tok]
            outst = const.tile([128, NBLK_PC, D], bf16)
            rec_sb = const.tile([1, H, TPC], bf16)        # softmax denom recips
            wu_sb = const.tile([1, NBT], bf16)
            ones_sb = const.tile([1, 128], bf16)
            nc.gpsimd.memset(ones_sb, 1.0)
            onesc_sb = const.tile([128, 1], bf16)
            nc.gpsimd.memset(onesc_sb, 1.0)
            nc.gpsimd.memset(v_sb[:, :, :, HD : HD + 1], 1.0)
            if VREM:
                # zero the key/value padding so padded scores are exp(0)*0
                # and padded V rows can't poison the AV accumulation
                nc.gpsimd.memset(k_sb[:, :, NBT:], 0.0)
                nc.gpsimd.memset(v_sb[VREM:128, NBC - 1, :, :], 0.0)
            # wu memset on the DVE so the warm-up can start ~0.3us in (the
            # Pool queue is busy with SWDGE desc-gen for the bias loads).
            nc.vector.memset(wu_sb, 0.0)

            # ---- PE warm-up -----------------------------------------------------
            # Dummy 512-row matmuls burn the p-state ramp (LOW/MID cycles)
            # while the first input DMAs are still in flight, so real matmuls
            # start at full speed.
            ps_wu = ps_bigp.tile([128, NBT], f32, tag="big", name="ps_wu")
            for _ in range(7):
                nc.tensor.matmul(
                    ps_wu[0:1, :], wu_sb[:, 0:1], wu_sb, start=True, stop=True
                )

            # ---- phase order: K -> Q(+scores h0-7) -> V(+scores h8-15) ---------
            def emit_scores(h):
                ptm = ptmp.tile([128, NBC, TPC], bf16, tag="ptm", name=f"ptm{h}")
                hp = (h % 2) * HD
                for blk in range(NBLK_PC):
                    ps_s = ps_sp.tile([128, NBC, 128], f32, tag="scores", name="ps_s")
                    for ci in range(NBC):
                        nc.tensor.matmul(
                            ps_s[:, ci, :],
                            k_sb[hp : hp + HD, h // 2, ts(ci, 128)],
                            q_sb[hp : hp + HD, h // 2, ts(blk, 128)],
                            start=True,
                            stop=True,
                        )
                    pt = ptp.tile([128, NBC, 128], bf16, tag="pt")
                    nc.scalar.activation(pt, ps_s, Exp, scale=float(SCALE))
                    nc.vector.tensor_mul(
                        ptm[:, :, ts(blk, 128)], pt, mask_sb[:, :, ts(blk, 128)]
                    )
                return ptm

            ptms = [None] * H
            av_tiles = [None] * H
            recb_tiles = [None] * (H // 2)
            op_ps = {}

            # ---- K projection: per-cc pass, kt-inner ---------------------------
            for cc in range(CC):
                ps = ps_bigp.tile([128, NBT], f32, tag="big", name="ps_k")
                for kt in range(KT):
                    nc.tensor.matmul(
                        ps,
                        wk_sb[:, cc, ts(kt, 128)],
                        xb_sb[:, kt, :],
                        start=(kt == 0),
                        stop=(kt == KT - 1),
                    )
                nc.vector.tensor_scalar_add(
                    k_sb[:, cc, 0:NBT], ps, bk_sb[:, cc : cc + 1]
                )

            # ---- Q projection + scores h0-7: one score-head per cc step --------
            # (one head per step keeps the Act exp queue fed at exactly the
            # rate the 2-deep scores PSUM ring can tolerate)
            for cc in range(CC):
                ps = ps_bigp.tile([128, TPC], f32, tag="big", name="ps_q")
                for kt in range(KT):
                    nc.tensor.matmul(
                        ps,
                        wq_sb[:, cc, ts(kt, 128)],
                        xb_sb[:, kt, 0:TPC],
                        start=(kt == 0),
                        stop=(kt == KT - 1),
                    )
                nc.vector.tensor_scalar_add(
                    q_sb[:, cc, :], ps, bq_sb[:, cc : cc + 1]
                )
                ptms[cc] = emit_scores(cc)

            # ---- V projection + scores h8-15 -----------------------------------
            for hf in range(2):
                for tc_ in range(NBC):
                    w = min(128, NBT - tc_ * 128)
                    ps = ps_bigp.tile([128, 512], f32, tag="big", name="ps_v")
                    for kt in range(KT):
                        nc.tensor.matmul(
                            ps[0:w, :],
                            xb_sb[:, kt, tc_ * 128 : tc_ * 128 + w],
                            wv_sb[:, kt, ts(hf, 512)],
                            start=(kt == 0),
                            stop=(kt == KT - 1),
                        )
                    nc.vector.tensor_add(
                        v_sb[0:w, tc_, ts(hf, 8), 0:HD],
                        ps[0:w, :].rearrange("p (h d) -> p h d", h=8),
                        bvb_sb[0:w, ts(hf, 512)].rearrange("p (h d) -> p h d", h=8),
                    )
                    g = 8 + hf * NBC + tc_
                    if g < H:
                        ptms[g] = emit_scores(g)

            for g in range(8 + 2 * NBC, H):
                ptms[g] = emit_scores(g)

            def emit_den_precompute():
                # Precompute the LAST pair's softmax denominators with a
                # ones-column matmul so their reciprocal/broadcast chain
                # completes long before the final evictions need it.
                for h in (H - 2, H - 1):
                    ps_den = ps_avp.tile([128, TPC], f32, tag="av", name="ps_den")
                    for ci in range(NBC):
                        nc.tensor.matmul(
                            ps_den[0:1, :],
                            onesc_sb,
                            ptms[h][:, ci, :],
                            start=(ci == 0),
                            stop=(ci == NBC - 1),
                        )
                    with nc.allow_low_precision(
                        reason="softmax denom reciprocal in bf16"
                    ):
                        nc.vector.reciprocal(rec_sb[:, h, :], ps_den[0:1, :])
                recb15 = smallp.tile([HD, 2, TPC], bf16, tag="recb")
                nc.gpsimd.partition_broadcast(recb15, rec_sb[:, H - 2 : H, :])
                recb_tiles[H // 2 - 1] = recb15

            # ---- transposed AV + fused normalize + out projection ---------------
            # Per head h: ps[0:65, :] = (V_h | ones)^T @ ptm_h   (both blocks)
            #             rec[h]      = 1 / ps[64, :]            (Act, PSUM->SB)
            #             recb        = bcast(rec[2j:2j+2])      (Pool, 2 heads/op)
            #             oT[h]       = ps[0:64, :] * recb       (DVE)
            # The o^T layout feeds the out-projection lhsT directly, and block
            # 0's out-projection matmuls interleave per-kt with the evictions
            # (oT column kt = heads 2kt,2kt+1) so the PE stays busy through
            # the eviction latency chains.
            def emit_avt(h):
                ps_av = ps_avp.tile([128, TPC], f32, tag="av", name="ps_av")
                for ci in range(NBC):
                    nc.tensor.matmul(
                        ps_av[0 : HD + 1, :],
                        v_sb[:, ci, h, :],
                        ptms[h][:, ci, :],
                        start=(ci == 0),
                        stop=(ci == NBC - 1),
                    )
                # Act copies the whole slab (o rows + denominator row) to
                # SBUF: the PSUM slot frees after this single op, and the
                # downstream reciprocal/multiply become cheap all-SBUF DVE
                # ops (4x mode).
                # reciprocal (DVE) and the o-slab copy (Act) read the PSUM
                # slot in parallel; the slot frees once both are done.
                # bf16 denominators: ~0.4% relative error, well inside the
                # 2e-2 budget. The last pair's denominators were precomputed
                # in the K phase.
                if h < H - 2:
                    with nc.allow_low_precision(
                        reason="softmax denom reciprocal in bf16"
                    ):
                        nc.vector.reciprocal(rec_sb[:, h, :], ps_av[HD : HD + 1, :])
                ou = smallp.tile([HD, TPC], bf16, tag="ou", name=f"ou{h}")
                nc.scalar.activation(ou, ps_av[0:HD, :], Ident)
                av_tiles[h] = ou
                if h % 2 == 1 and h < H - 2:
                    recb = smallp.tile([HD, 2, TPC], bf16, tag="recb")
                    nc.gpsimd.partition_broadcast(recb, rec_sb[:, h - 1 : h + 1, :])
                    recb_tiles[h // 2] = recb

            def emit_evict(h):
                nc.vector.tensor_mul(
                    oT_sb[(h % 2) * HD : (h % 2) * HD + HD, h // 2, :],
                    av_tiles[h][0:HD, :],
                    recb_tiles[h // 2][:, h % 2, :],
                )

            def emit_op_mm(blk, kt, nbs=(0, 1)):
                # each group's first matmul is a K=1 ones-row matmul that
                # seeds the accumulator with the output bias, so the final
                # result can be DMAed straight out of PSUM (no DVE add).
                for nb in nbs:
                    if (blk, nb) not in op_ps:
                        pool = ps_sp if (blk, nb) == (1, 1) else ps_bigp
                        tag = "scores" if (blk, nb) == (1, 1) else "big"
                        ps = pool.tile([128, 512], f32, tag=tag, name="ps_o")
                        op_ps[(blk, nb)] = ps
                        nc.tensor.matmul(
                            ps,
                            ones_sb,
                            boutr_sb[:, ts(nb, 512)],
                            start=True,
                            stop=False,
                        )
                    nc.tensor.matmul(
                        op_ps[(blk, nb)],
                        oT_sb[:, kt, ts(blk, 128)],
                        wout_sb[:, kt, ts(nb, 512)],
                        start=False,
                        stop=(kt == KT - 1),
                    )

            def emit_op_fin(blk, nb):
                # bias already folded in via the ones-row matmul; the bf16
                # eviction copies alternate Act/DVE so they pipeline
                # two-wide, and one whole-block bf16 store per block keeps
                # the tail to two HWDGE slots (~0.4% output rounding, well
                # in budget).
                ps = op_ps.pop((blk, nb))
                dst = outst[:, blk, ts(nb, 512)]
                if nb == 0:
                    nc.scalar.activation(dst, ps, Ident)
                else:
                    with nc.allow_low_precision(reason="bf16 output store"):
                        nc.vector.tensor_copy(dst, ps)
                if not any(k[0] == blk for k in op_ps):
                    nc.scalar.dma_start(out_d[ts(blk, 128), :], outst[:, blk, :])

            # Three out-projection accumulations (blk0 nb0/nb1, blk1 nb0)
            # ride along with the evictions per-kt; only blk1 nb1's eight
            # matmuls remain in the serial tail.
            for h in range(H):
                emit_avt(h)
                if h == 2:
                    emit_den_precompute()
                if h >= 3 and h % 2 == 1:
                    emit_evict(h - 3)
                    emit_evict(h - 2)
                    if h >= 5:
                        kt = (h - 5) // 2
                        emit_op_mm(0, kt)
                        emit_op_mm(1, kt)
            for h in (H - 2, H - 1):
                emit_evict(h)
            emit_op_mm(0, 6)
            emit_op_mm(1, 6)
            emit_op_mm(0, 7)
            emit_op_fin(0, 0)
            emit_op_fin(0, 1)
            emit_op_mm(1, 7)
            emit_op_fin(1, 0)
            emit_op_fin(1, 1)
            assert not op_ps

    nc.compile()
    return nc


_prog_cache = {}


def _get_program(nbt):
    key = int(nbt)
    if key not in _prog_cache:
        _prog_cache[key] = _build_program(key)
    return _prog_cache[key]


def _routing(cp):
    """Exact reference routing (top_k tie behaviour included) + band layout.

    Returns (order, S, NBC, masks): sorted order, per-core band start chunk,
    band width in 128-chunks, and per-core [128, NBC, 256] 0/1 masks.
    """
    dist = np.abs(cp[:, None] - cp[None, :])
    routes = np.argsort(dist, axis=1, kind="stable")[:, :K_NEIGH]
    order = np.argsort(cp, kind="stable")
    rank = np.empty(N, np.int64)
    rank[order] = np.arange(N)

    kr = rank[routes[order]]  # [N(sorted q), K] key ranks per sorted query
    core_lo = kr.min(axis=1).reshape(NCORES, TPC).min(axis=1)
    core_hi = kr.max(axis=1).reshape(NCORES, TPC).max(axis=1)
    nbt = int((core_hi - core_lo + 1).max())
    nbt = max((nbt + 63) // 64 * 64, 320)
    if nbt > MAX_NBC * 128:
        raise AssertionError(f"kNN band needs {nbt} tokens > cap {MAX_NBC * 128}")
    nbc = (nbt + 127) // 128
    # 64-granular band start, clamped so the window stays inside [0, N)
    S64 = np.minimum(core_lo // 64, (N - nbt) // 64).astype(np.int64)
    assert (core_hi < S64 * 64 + nbt).all()
    # Rotation: own tokens [256c, 256c+256) move to band columns [0, TPC)
    roll = TPC * np.arange(NCORES) - S64 * 64
    assert (roll >= 0).all() and (roll + TPC <= nbt).all()
    masks = np.zeros((NCORES, 128, nbc, TPC), np.float32)
    qloc = np.broadcast_to((np.arange(N) % TPC)[:, None], kr.shape)
    corei = np.broadcast_to((np.arange(N) // TPC)[:, None], kr.shape)
    rel = kr - S64[corei] * 64
    assert rel.min() >= 0 and rel.max() < nbt
    rel = (rel - roll[corei]) % nbt
    masks[corei, rel % 128, rel // 128, qloc] = 1.0
    return order, S64, roll, nbt, masks


def _make_in_maps(x, cantor_positions, W_qkv, b_qkv, W_out, b_out):
    x = np.asarray(x, np.float32)
    cp = np.asarray(cantor_positions, np.float32)
    W_qkv = np.asarray(W_qkv, np.float32)
    b_qkv = np.asarray(b_qkv, np.float32)
    W_out = np.asarray(W_out, np.float32)
    b_out = np.asarray(b_out, np.float32)
    assert x.shape == (1, N, D)

    order, S64, roll, nbt, masks = _routing(cp)

    xsT = np.ascontiguousarray(x[0][order].T)                    # [D, N] f32

    def cc_swizzle(w):
        # [D, D] -> [CC, 128, KT*128]: w[kt*128+p, cc*128+c] -> out[cc, p, kt*128+c]
        return np.ascontiguousarray(
            w.reshape(KT, 128, CC, 128).transpose(2, 1, 0, 3).reshape(CC, 128, KT * 128)
        ).astype(BF16)

    wq_s = cc_swizzle(W_qkv[:, 0:D])
    wk_s = cc_swizzle(W_qkv[:, D : 2 * D])
    wv_s = np.ascontiguousarray(W_qkv[:, 2 * D : 3 * D]).astype(BF16)
    wout_s = np.ascontiguousarray(W_out).astype(BF16)
    bq_s = np.ascontiguousarray(b_qkv[0:D].reshape(CC, 128).T, np.float32)
    bk_s = np.ascontiguousarray(b_qkv[D : 2 * D].reshape(CC, 128).T, np.float32)
    bv_s = np.ascontiguousarray(b_qkv[2 * D : 3 * D], np.float32)
    boutr_s = np.ascontiguousarray(b_out.reshape(1, D)).astype(BF16)

    in_maps = []
    for c in range(NCORES):
        xb = xsT[:, 64 * S64[c] : 64 * S64[c] + nbt]
        xb = np.roll(xb, -int(roll[c]), axis=1)  # own tokens -> cols 0:TPC
        in_maps.append(
            {
                "xb": np.ascontiguousarray(xb).astype(BF16),
                "wq": wq_s,
                "wk": wk_s,
                "wv": wv_s,
                "wout": wout_s,
                "mask": masks[c].astype(BF16),
                "bq": bq_s,
                "bk": bk_s,
                "bv": bv_s,
                "boutr": boutr_s,
            }
        )
    return order, nbt, in_maps


def kernel(x, cantor_positions, W_qkv, b_qkv, W_out, b_out):
    global LAST_RESULT
    order, nbt, in_maps = _make_in_maps(
        x, cantor_positions, W_qkv, b_qkv, W_out, b_out
    )
    nc = _get_program(nbt)

    res = run_bass_kernel_spmd(nc, in_maps, list(range(NCORES)))
    LAST_RESULT = res

    out_sorted = np.concatenate(
        [res.results[c]["out"].astype(np.float32) for c in range(NCORES)], axis=0
    )
    final = np.empty((N, D), np.float32)
    final[order] = out_sorted
    return final.reshape(1, N, D)
